# revision 12
# baseline (speedup 1.0000x reference)
"""GCN 3-layer message passing kernel for Trainium2 (8 NeuronCores).

Sharding: nodes relabeled by (owner core, degree rank); core c owns 12500
consecutive new-ids and segment-sums messages for its own dst nodes. The
node-feature table (64-float padded rows, dinv folded in) is rebuilt on
every core via AllGather each layer. Gathers are hybrid: the uniform base
of each (tile, bank) grid is fetched with batched ant dma_gather (int16
indices, 512 tokens/instr, spread over 4 SWDGE queues); the ragged
overflow uses per-column indirect DMA on queue 0. Host does the layer-1
input transform (x@W1*dinv), the final pooling and the linear head.
"""

import time
import numpy as np

N_NODES = 100000
N_EDGES = 3200000
FEAT = 30
HID = 30
N_GRAPHS = 512
NCORES = 8
NODES_PER_CORE = N_NODES // NCORES  # 12500
P = 128
NTILES = (NODES_PER_CORE + P - 1) // P  # 98 (last tile holds 84 nodes)
SLAB = NTILES * P                       # 12544 padded rows per core slab
GROWS = NCORES * SLAB                   # 100352 padded table rows
ROWW = 64                               # padded row width (floats, 256B)
BANKW = 32768                           # int16 index window
NB = (GROWS + BANKW - 1) // BANKW       # 4 banks
CHUNK = 512                             # tokens per dma_gather instr
PAD_ROW = NODES_PER_CORE                # row 12500: zero row of slab 0

# per-bank zero rows (slab tails), as bank-local int16 values
_ZROWS = []
for _b in range(NB):
    lo = _b * BANKW
    hi = min(lo + BANKW, GROWS)
    z = None
    for _c in range(NCORES):
        zr = _c * SLAB + NODES_PER_CORE
        if lo <= zr < hi:
            z = zr - lo
            break
    assert z is not None
    _ZROWS.append(z)

_COMPILED = None
_COMPILED_KEY = None


class _Runner:
    """Compile a Bacc kernel once; run it on NCORES cores via PJRT."""

    def __init__(self, nc, n_cores):
        import jax
        import concourse.mybir as mybir
        from concourse.bass2jax import (
            _bass_exec_p, install_neuronx_cc_hook, partition_id_tensor)
        from jax.sharding import Mesh, PartitionSpec
        from jax.experimental.shard_map import shard_map

        install_neuronx_cc_hook()
        self.jax = jax
        self.n_cores = n_cores
        partition_name = (nc.partition_id_tensor.name
                          if nc.partition_id_tensor else None)
        in_names, out_names, out_avals, zero_outs = [], [], [], []
        for alloc in nc.m.functions[0].allocations:
            if not isinstance(alloc, mybir.MemoryLocationSet):
                continue
            name = alloc.memorylocations[0].name
            if alloc.kind == "ExternalInput":
                if name != partition_name:
                    in_names.append(name)
            elif alloc.kind == "ExternalOutput":
                shape = tuple(alloc.tensor_shape)
                dtype = mybir.dt.np(alloc.dtype)
                out_names.append(name)
                out_avals.append(jax.core.ShapedArray(shape, dtype))
                zero_outs.append(np.zeros(shape, dtype))
        self.in_names, self.out_names, self.zero_outs = (
            in_names, out_names, zero_outs)
        n_params, n_outs = len(in_names), len(out_avals)
        all_in_names = in_names + out_names + (
            [partition_name] if partition_name else [])

        def _body(*args):
            operands = list(args)
            if partition_name is not None:
                operands.append(partition_id_tensor())
            return tuple(_bass_exec_p.bind(
                *operands,
                out_avals=tuple(out_avals),
                in_names=tuple(all_in_names),
                out_names=tuple(out_names),
                lowering_input_output_aliases=(),
                sim_require_finite=True,
                sim_require_nnan=True,
                nc=nc,
            ))

        try:
            devices = jax.devices("axon")[:n_cores]
        except RuntimeError:
            devices = jax.devices()[:n_cores]
        mesh = Mesh(np.asarray(devices), ("core",))
        self.fn = jax.jit(
            shard_map(_body, mesh=mesh,
                      in_specs=(PartitionSpec("core"),) * (n_params + n_outs),
                      out_specs=(PartitionSpec("core"),) * n_outs,
                      check_rep=False),
            keep_unused=True,
        )

    def put_inputs(self, in_maps):
        per_core = [[np.asarray(m[name]) for name in self.in_names]
                    for m in in_maps]
        concat_in = [
            np.concatenate([per_core[c][i] for c in range(self.n_cores)],
                           axis=0)
            for i in range(len(self.in_names))
        ]
        self.dev_in = [self.jax.device_put(a) for a in concat_in]
        self.dev_zo = [self.jax.device_put(z) for z in self._zo()]

    def _zo(self):
        return [np.concatenate([z] * self.n_cores, axis=0)
                for z in self.zero_outs]

    def call(self):
        res = self.fn(*self.dev_in, *self.dev_zo)
        self.jax.block_until_ready(res)
        return res

    def burst(self, burst=10):
        self.call()
        t0 = time.time()
        res = None
        for _ in range(burst):
            res = self.fn(*self.dev_in, *self.dev_zo)
        self.jax.block_until_ready(res)
        return (time.time() - t0) / burst

    def results(self, res):
        out = []
        for c in range(self.n_cores):
            d = {}
            for i, name in enumerate(self.out_names):
                full = np.asarray(res[i])
                sz = full.shape[0] // self.n_cores
                d[name] = full[c * sz:(c + 1) * sz]
            out.append(d)
        return out


def _rank_within_groups(g, n_groups):
    """rank of each element within its group; g must be sorted ascending."""
    gstart = np.searchsorted(g, np.arange(n_groups), side="left")
    return np.arange(len(g)) - gstart[g]


def _build_schedule(edge_index, w_ant=999.0, w_ind=8.4):
    src = np.asarray(edge_index[0], dtype=np.int64)
    dst = np.asarray(edge_index[1], dtype=np.int64)

    deg = np.bincount(dst, minlength=N_NODES).astype(np.int64) + 1
    dinv = (1.0 / np.sqrt(np.maximum(deg, 1).astype(np.float64))).astype(np.float32)

    # owner core by round-robin over degree rank; within a core nodes are
    # degree-sorted so each 128-node tile has near-uniform degree
    order = np.argsort(-deg, kind="stable")
    perm = np.empty(N_NODES, dtype=np.int64)
    for c in range(NCORES):
        perm[c * NODES_PER_CORE:(c + 1) * NODES_PER_CORE] = order[c::NCORES]
    inv_perm = np.empty(N_NODES, dtype=np.int64)
    inv_perm[perm] = np.arange(N_NODES)

    nsrc = inv_perm[src]
    ndst = inv_perm[dst]

    gp = (nsrc // NODES_PER_CORE) * SLAB + (nsrc % NODES_PER_CORE)
    bank = gp // BANKW
    r16 = (gp - bank * BANKW).astype(np.int64)

    core = ndst // NODES_PER_CORE
    loc = ndst % NODES_PER_CORE
    tile = loc // P
    part = loc % P

    # sort edges by (core, tile, bank, part)
    order_e = np.lexsort((gp, part, bank, tile, core))
    core_s, tile_s, bank_s, part_s = (core[order_e], tile[order_e],
                                      bank[order_e], part[order_e])
    gp_s, r16_s = gp[order_e], r16[order_e]
    g = (((core_s * NTILES + tile_s) * NB + bank_s) * P + part_s)
    ngroups = NCORES * NTILES * NB * P
    cnt = np.bincount(g, minlength=ngroups).reshape(NCORES, NTILES, NB, P)
    rank = _rank_within_groups(g, ngroups)

    # choose K per (tile, bank): tokens gathered via ant path per slot
    Ks = np.arange(0, 28, 4)
    # cost[t,b,K] = ant tokens (always K per slot) + weighted overflow
    c_flat = cnt.transpose(1, 2, 0, 3).reshape(NTILES, NB, NCORES * P)
    costs = np.stack([
        w_ant * Kc * (NCORES * P)
        + w_ind * np.maximum(c_flat - Kc, 0).sum(axis=2)
        for Kc in Ks], axis=2)  # [NT, NB, nK]
    K = Ks[costs.argmin(axis=2)]  # [NT, NB]

    ni_tb = K // 4                           # instrs per (tile, bank)
    n_instr = int(ni_tb.sum())
    instr_base = np.zeros((NTILES, NB), dtype=np.int64)
    run = 0
    bank_of_instr = np.empty(n_instr, dtype=np.int64)
    for t in range(NTILES):
        for b in range(NB):
            instr_base[t, b] = run
            bank_of_instr[run:run + ni_tb[t, b]] = b
            run += ni_tb[t, b]
    cols_total = n_instr * 32

    # ant idx array [NCORES, 16, cols_total] int16, then replicate to 128
    zr = np.asarray(_ZROWS, dtype=np.int16)
    arr = np.empty((NCORES, 16, cols_total), dtype=np.int16)
    arr[:, :, :] = np.repeat(zr[bank_of_instr], 32)[None, None, :]

    Ke = K[tile_s, bank_s]
    sel = rank < Ke
    I = instr_base[tile_s[sel], bank_s[sel]] + rank[sel] // 4
    k = (rank[sel] % 4) * P + part_s[sel]
    c16 = I * 32 + k // 16
    r_in = k % 16
    arr[core_s[sel], r_in, c16] = r16_s[sel].astype(np.int16)
    idx_arr = np.tile(arr, (1, 8, 1))  # [NCORES, 128, cols_total]

    # overflow edges -> per (core, tile, part) column lists
    ov = ~sel
    core_o, tile_o, part_o, gp_o = (core_s[ov], tile_s[ov], part_s[ov],
                                    gp_s[ov])
    g2 = (core_o * NTILES + tile_o) * P + part_o
    # already sorted by (core, tile, bank, part) -> re-sort by (core,tile,part)
    o2 = np.lexsort((g2,))
    g2 = g2[o2]
    gp_o = gp_o[o2]
    part_o2 = part_o[o2]
    tile_o2 = tile_o[o2]
    core_o2 = core_o[o2]
    n2 = NCORES * NTILES * P
    ocnt = np.bincount(g2, minlength=n2).reshape(NCORES, NTILES, P)
    orank = _rank_within_groups(g2, n2)
    D_ov = ocnt.max(axis=(0, 2)).astype(np.int64)  # [NT]

    ov_base = np.concatenate([[0], np.cumsum(P * D_ov)])
    total_ov = int(ov_base[-1])
    offs = np.full((NCORES, max(total_ov, P)), PAD_ROW, dtype=np.int32)
    slot = ov_base[tile_o2] + orank * P + part_o2
    offs[core_o2, slot] = gp_o.astype(np.int32)

    return {
        "perm": perm, "dinv": dinv,
        "K": K, "instr_base": instr_base, "n_instr": n_instr,
        "cols_total": cols_total, "idx_arr": idx_arr,
        "D_ov": D_ov, "ov_base": ov_base, "total_ov": max(total_ov, P),
        "offs": offs,
    }


def _build_program(K, instr_base, n_instr, cols_total, D_ov, ov_base,
                   total_ov):
    import concourse.bass as bass
    import concourse.bacc as bacc
    import concourse.mybir as mybir
    from concourse.tile import TileContext
    from concourse.masks import make_identity
    from concourse import library_config

    fp32 = mybir.dt.float32
    i16 = mybir.dt.int16
    nc = bacc.Bacc("TRN2", target_bir_lowering=False, debug=False,
                   num_devices=NCORES, num_swdge_queues=4)

    g1own = nc.dram_tensor("g1own", [SLAB, FEAT], fp32, kind="ExternalInput").ap()
    idxs_in = nc.dram_tensor("idxs", [P, max(cols_total, 32)], i16,
                             kind="ExternalInput").ap()
    offsets = nc.dram_tensor("offsets", [total_ov], mybir.dt.int32,
                             kind="ExternalInput").ap()
    dinv_in = nc.dram_tensor("dinv", [SLAB, 1], fp32, kind="ExternalInput").ap()
    w2 = nc.dram_tensor("w2", [HID, HID], fp32, kind="ExternalInput").ap()
    w3 = nc.dram_tensor("w3", [HID, HID], fp32, kind="ExternalInput").ap()
    bb = nc.dram_tensor("bb", [P, 3, HID], fp32, kind="ExternalInput").ap()
    h3_out = nc.dram_tensor("h3", [SLAB, HID], fp32, kind="ExternalOutput").ap()

    gown = nc.dram_tensor("gown", [SLAB, ROWW], fp32)
    gfull = nc.dram_tensor("gfull", [GROWS, ROWW], fp32, addr_space="Shared")

    nfull = NODES_PER_CORE // P          # 97 full tiles
    nrem = NODES_PER_CORE - nfull * P    # 84

    # static per-tile gather plans
    TOT = [int(K[t].sum() + D_ov[t]) for t in range(NTILES)]

    # greedy queue assignment: indirect is pinned to q0; ants balance the rest
    ant_cost = 1.0 + 0.0082 * CHUNK
    ind_cost = 1.05
    qload = [0.0, 0.0, 0.0, 0.0]
    qload[0] += ind_cost * float(sum(int(D_ov[t]) for t in range(NTILES)))
    ant_q = []
    for t in range(NTILES):
        for b in range(NB):
            for i in range(int(K[t, b]) // 4):
                qi = min(range(1, 4), key=lambda q: qload[q])
                qload[qi] += ant_cost
                ant_q.append(qi)

    with TileContext(nc) as tc:
        with (
            tc.tile_pool(name="const", bufs=1) as cp,
            tc.tile_pool(name="stageA", bufs=1) as stA,
            tc.tile_pool(name="stageB", bufs=1) as stB,
            tc.tile_pool(name="work", bufs=3) as wp,
            tc.tile_pool(name="small", bufs=6) as sp,
            tc.tile_pool(name="psumT", bufs=2, space="PSUM") as ppT,
            tc.tile_pool(name="psumG", bufs=2, space="PSUM") as ppG,
        ):
            ident = cp.tile([P, P], fp32)
            make_identity(nc, ident[:])
            nc.gpsimd.load_library(library_config.mlp)
            w2t = cp.tile([HID, HID], fp32)
            nc.sync.dma_start(out=w2t[:], in_=w2[:, :])
            w3t = cp.tile([HID, HID], fp32)
            nc.sync.dma_start(out=w3t[:], in_=w3[:, :])
            bbt = cp.tile([P, 3, HID], fp32)
            nc.sync.dma_start(out=bbt[:], in_=bb[:, :, :])
            dinv_t = cp.tile([P, NTILES], fp32)
            nc.sync.dma_start(
                out=dinv_t[:],
                in_=dinv_in[:, 0].rearrange("(t p) -> p t", p=P),
            )
            ncols_ov = total_ov // P
            offs_all = cp.tile([P, ncols_ov], mybir.dt.int32)
            nc.sync.dma_start(
                out=offs_all[:],
                in_=offsets[:].rearrange("(d p) -> p d", p=P),
            )
            idxs_all = cp.tile([P, max(cols_total, 32)], i16)
            nc.sync.dma_start(out=idxs_all[:], in_=idxs_in[:, :])

            # zero gown once (rows >= 12500 and cols >= 30 stay zero forever)
            zt = cp.tile([P, 16, ROWW], fp32)
            nc.vector.memset(zt[:], 0.0)
            for k0 in range(0, NTILES, 16):
                k1 = min(k0 + 16, NTILES)
                nc.sync.dma_start(
                    out=gown[k0 * P:k1 * P, :].rearrange(
                        "(t p) f -> p t f", p=P),
                    in_=zt[:, :k1 - k0, :],
                )

            stage = stA.tile([P, NTILES, HID], fp32)
            stage2 = stB.tile([P, NTILES, HID], fp32)
            nc.sync.dma_start(
                out=stage[:],
                in_=g1own[:, :].rearrange("(t p) f -> p t f", p=P),
            )

            tc.strict_bb_all_engine_barrier()
            r512 = nc.gpsimd.to_reg(CHUNK)

            def publish(st):
                nc.sync.dma_start(
                    out=gown[:nfull * P, :FEAT].rearrange(
                        "(t p) f -> p t f", p=P),
                    in_=st[:, :nfull, :],
                )
                if nrem:
                    nc.sync.dma_start(
                        out=gown[nfull * P:NODES_PER_CORE, :FEAT],
                        in_=st[:nrem, nfull, :],
                    )
                tc.strict_bb_all_engine_barrier()
                nc.gpsimd.collective_compute(
                    "AllGather", mybir.AluOpType.bypass,
                    replica_groups=[list(range(NCORES))],
                    ins=[gown[:, :]], outs=[gfull[:, :]],
                )
                tc.strict_bb_all_engine_barrier()

            publish(stage)

            def compute_tile(t, layer, msg, cur_stage, nxt_stage):
                s0 = sp.tile([P, HID], fp32, tag="s0")
                nc.vector.tensor_reduce(
                    out=s0[:],
                    in_=msg[:, :, :FEAT].rearrange("p d f -> p f d"),
                    axis=mybir.AxisListType.X, op=mybir.AluOpType.add,
                )
                s1 = sp.tile([P, HID], fp32, tag="s1")
                nc.vector.tensor_add(
                    out=s1[:], in0=cur_stage[:, t, :], in1=s0[:])
                s2 = sp.tile([P, HID], fp32, tag="s2")
                nc.vector.scalar_tensor_tensor(
                    out=s2[:], in0=s1[:], scalar=dinv_t[:, t:t + 1],
                    in1=bbt[:, layer, :],
                    op0=mybir.AluOpType.mult, op1=mybir.AluOpType.add,
                )
                h = sp.tile([P, HID], fp32, tag="h")
                nc.scalar.activation(
                    h[:], s2[:], mybir.ActivationFunctionType.Relu)
                if layer < 2:
                    ht_ps = ppT.tile([HID, P], fp32, tag="tps")
                    nc.tensor.transpose(out=ht_ps[:], in_=h[:],
                                        identity=ident[:])
                    ht = sp.tile([HID, P], fp32, tag="ht")
                    nc.vector.tensor_copy(out=ht[:], in_=ht_ps[:])
                    g_ps = ppG.tile([P, HID], fp32, tag="gps")
                    wmat = w2t if layer == 0 else w3t
                    nc.tensor.matmul(out=g_ps[:], lhsT=ht[:], rhs=wmat[:],
                                     start=True, stop=True)
                    nc.vector.tensor_scalar_mul(
                        out=nxt_stage[:, t, :], in0=g_ps[:],
                        scalar1=dinv_t[:, t:t + 1])
                else:
                    nc.vector.tensor_copy(out=nxt_stage[:, t, :], in_=h[:])

            cur_stage, nxt_stage = stage, stage2
            for layer in range(3):
                ant_i = 0
                for t in range(NTILES):
                    msg = wp.tile([P, max(TOT[t], 1), ROWW], fp32,
                                  tag="msg", name="msg")
                    col = 0
                    for b in range(NB):
                        lo = b * BANKW
                        hi = min(lo + BANKW, GROWS)
                        for i in range(int(K[t, b]) // 4):
                            ioff = int(instr_base[t, b] + i) * 32
                            nc.gpsimd.dma_gather(
                                msg[:, col:col + 4, :],
                                gfull[lo:hi, :],
                                idxs_all[:, ioff:ioff + 32],
                                CHUNK, r512, ROWW,
                                single_packet=False,
                                queue_num=ant_q[ant_i],
                            )
                            ant_i += 1
                            col += 4
                    cbase = int(ov_base[t]) // P
                    for j in range(int(D_ov[t])):
                        nc.gpsimd.indirect_dma_start(
                            out=msg[:, col + j, :FEAT],
                            out_offset=None,
                            in_=gfull[:, :],
                            in_offset=bass.IndirectOffsetOnAxis(
                                ap=offs_all[:, cbase + j:cbase + j + 1],
                                axis=0),
                        )
                    compute_tile(t, layer, msg, cur_stage, nxt_stage)
                if layer < 2:
                    publish(nxt_stage)
                cur_stage, nxt_stage = nxt_stage, cur_stage

            nc.sync.dma_start(
                out=h3_out[:, :].rearrange("(t p) f -> p t f", p=P),
                in_=cur_stage[:],
            )

    nc.compile()
    return nc


def kernel(x, edge_index, batch_ids, W1, b1, W2, b2, W3, b3, lin_W, lin_b):
    global _COMPILED, _COMPILED_KEY
    x = np.asarray(x, dtype=np.float32)
    edge_index = np.asarray(edge_index)
    batch_ids = np.asarray(batch_ids)
    W1 = np.asarray(W1, np.float32); b1 = np.asarray(b1, np.float32)
    W2 = np.asarray(W2, np.float32); b2 = np.asarray(b2, np.float32)
    W3 = np.asarray(W3, np.float32); b3 = np.asarray(b3, np.float32)
    lin_W = np.asarray(lin_W, np.float32); lin_b = np.asarray(lin_b, np.float32)

    sched = _build_schedule(edge_index)
    perm, dinv = sched["perm"], sched["dinv"]

    key = (sched["K"].tobytes() + sched["D_ov"].tobytes()
           + np.int64(sched["cols_total"]).tobytes())
    if _COMPILED is None or _COMPILED_KEY != key:
        nc = _build_program(sched["K"], sched["instr_base"],
                            sched["n_instr"], sched["cols_total"],
                            sched["D_ov"], sched["ov_base"],
                            sched["total_ov"])
        _COMPILED = _Runner(nc, NCORES)
        _COMPILED_KEY = key
    r = _COMPILED

    g1 = (x @ W1) * dinv[:, None]
    g1p = g1[perm]
    dinvp = dinv[perm]

    bbc = np.stack([
        np.broadcast_to(b1, (P, HID)),
        np.broadcast_to(b2, (P, HID)),
        np.broadcast_to(b3, (P, HID)),
    ], axis=1).astype(np.float32)  # [P, 3, HID]
    cols_total = sched["cols_total"]
    in_maps = []
    for c in range(NCORES):
        lo, hi = c * NODES_PER_CORE, (c + 1) * NODES_PER_CORE
        g1own = np.zeros((SLAB, FEAT), np.float32)
        g1own[:NODES_PER_CORE] = g1p[lo:hi]
        dv = np.zeros((SLAB, 1), np.float32)
        dv[:NODES_PER_CORE, 0] = dinvp[lo:hi]
        idxs = sched["idx_arr"][c]
        if cols_total < 32:
            pad = np.zeros((P, 32), np.int16)
            pad[:, :cols_total] = idxs
            idxs = pad
        in_maps.append({
            "g1own": g1own,
            "idxs": idxs,
            "offsets": sched["offs"][c],
            "dinv": dv,
            "w2": W2, "w3": W3, "bb": bbc,
        })

    r.put_inputs(in_maps)
    res = r.call()
    results = r.results(res)

    h3p = np.concatenate(
        [results[c]["h3"][:NODES_PER_CORE] for c in range(NCORES)], axis=0)
    h3 = np.empty_like(h3p)
    h3[perm] = h3p
    pooled = np.zeros((N_GRAPHS, HID), np.float32)
    np.add.at(pooled, batch_ids.astype(np.int64), h3)
    return pooled @ lin_W + lin_b


# revision 13
# speedup vs baseline: 1.0905x; 1.0905x over previous
"""GCN 3-layer message passing kernel for Trainium2 (8 NeuronCores).

Sharding: nodes relabeled by (owner core, degree rank); core c owns 12500
consecutive new-ids and segment-sums messages for its own dst nodes. The
node-feature table (64-float padded rows, dinv folded in) is rebuilt on
every core via AllGather each layer. Gathers are hybrid: the uniform base
of each (tile, bank) grid is fetched with batched ant dma_gather (int16
indices, 512 tokens/instr, spread over 4 SWDGE queues); the ragged
overflow uses per-column indirect DMA on queue 0. Host does the layer-1
input transform (x@W1*dinv), the final pooling and the linear head.
"""

import time
import numpy as np

N_NODES = 100000
N_EDGES = 3200000
FEAT = 30
HID = 30
N_GRAPHS = 512
NCORES = 8
NODES_PER_CORE = N_NODES // NCORES  # 12500
P = 128
NTILES = (NODES_PER_CORE + P - 1) // P  # 98 (last tile holds 84 nodes)
SLAB = NTILES * P                       # 12544 padded rows per core slab
GROWS = NCORES * SLAB                   # 100352 padded table rows
ROWW = 64                               # padded row width (floats, 256B)
BANKW = 32768                           # int16 index window
NB = (GROWS + BANKW - 1) // BANKW       # 4 banks
CHUNK = 512                             # tokens per dma_gather instr
PAD_ROW = NODES_PER_CORE                # row 12500: zero row of slab 0

# per-bank zero rows (slab tails), as bank-local int16 values
_ZROWS = []
for _b in range(NB):
    lo = _b * BANKW
    hi = min(lo + BANKW, GROWS)
    z = None
    for _c in range(NCORES):
        zr = _c * SLAB + NODES_PER_CORE
        if lo <= zr < hi:
            z = zr - lo
            break
    assert z is not None
    _ZROWS.append(z)

_COMPILED = None
_COMPILED_KEY = None


class _Runner:
    """Compile a Bacc kernel once; run it on NCORES cores via PJRT."""

    def __init__(self, nc, n_cores):
        import jax
        import concourse.mybir as mybir
        from concourse.bass2jax import (
            _bass_exec_p, install_neuronx_cc_hook, partition_id_tensor)
        from jax.sharding import Mesh, PartitionSpec
        from jax.experimental.shard_map import shard_map

        install_neuronx_cc_hook()
        self.jax = jax
        self.n_cores = n_cores
        partition_name = (nc.partition_id_tensor.name
                          if nc.partition_id_tensor else None)
        in_names, out_names, out_avals, zero_outs = [], [], [], []
        for alloc in nc.m.functions[0].allocations:
            if not isinstance(alloc, mybir.MemoryLocationSet):
                continue
            name = alloc.memorylocations[0].name
            if alloc.kind == "ExternalInput":
                if name != partition_name:
                    in_names.append(name)
            elif alloc.kind == "ExternalOutput":
                shape = tuple(alloc.tensor_shape)
                dtype = mybir.dt.np(alloc.dtype)
                out_names.append(name)
                out_avals.append(jax.core.ShapedArray(shape, dtype))
                zero_outs.append(np.zeros(shape, dtype))
        self.in_names, self.out_names, self.zero_outs = (
            in_names, out_names, zero_outs)
        n_params, n_outs = len(in_names), len(out_avals)
        all_in_names = in_names + out_names + (
            [partition_name] if partition_name else [])

        def _body(*args):
            operands = list(args)
            if partition_name is not None:
                operands.append(partition_id_tensor())
            return tuple(_bass_exec_p.bind(
                *operands,
                out_avals=tuple(out_avals),
                in_names=tuple(all_in_names),
                out_names=tuple(out_names),
                lowering_input_output_aliases=(),
                sim_require_finite=True,
                sim_require_nnan=True,
                nc=nc,
            ))

        try:
            devices = jax.devices("axon")[:n_cores]
        except RuntimeError:
            devices = jax.devices()[:n_cores]
        mesh = Mesh(np.asarray(devices), ("core",))
        self.fn = jax.jit(
            shard_map(_body, mesh=mesh,
                      in_specs=(PartitionSpec("core"),) * (n_params + n_outs),
                      out_specs=(PartitionSpec("core"),) * n_outs,
                      check_rep=False),
            keep_unused=True,
        )

    def put_inputs(self, in_maps):
        per_core = [[np.asarray(m[name]) for name in self.in_names]
                    for m in in_maps]
        concat_in = [
            np.concatenate([per_core[c][i] for c in range(self.n_cores)],
                           axis=0)
            for i in range(len(self.in_names))
        ]
        self.dev_in = [self.jax.device_put(a) for a in concat_in]
        self.dev_zo = [self.jax.device_put(z) for z in self._zo()]

    def _zo(self):
        return [np.concatenate([z] * self.n_cores, axis=0)
                for z in self.zero_outs]

    def call(self):
        res = self.fn(*self.dev_in, *self.dev_zo)
        self.jax.block_until_ready(res)
        return res

    def burst(self, burst=10):
        self.call()
        t0 = time.time()
        res = None
        for _ in range(burst):
            res = self.fn(*self.dev_in, *self.dev_zo)
        self.jax.block_until_ready(res)
        return (time.time() - t0) / burst

    def results(self, res):
        out = []
        for c in range(self.n_cores):
            d = {}
            for i, name in enumerate(self.out_names):
                full = np.asarray(res[i])
                sz = full.shape[0] // self.n_cores
                d[name] = full[c * sz:(c + 1) * sz]
            out.append(d)
        return out


def _rank_within_groups(g, n_groups):
    """rank of each element within its group; g must be sorted ascending."""
    gstart = np.searchsorted(g, np.arange(n_groups), side="left")
    return np.arange(len(g)) - gstart[g]


def _build_schedule(edge_index, w_ant=7.0, w_ind=8.4):
    src = np.asarray(edge_index[0], dtype=np.int64)
    dst = np.asarray(edge_index[1], dtype=np.int64)

    deg = np.bincount(dst, minlength=N_NODES).astype(np.int64) + 1
    dinv = (1.0 / np.sqrt(np.maximum(deg, 1).astype(np.float64))).astype(np.float32)

    # owner core by round-robin over degree rank; within a core nodes are
    # degree-sorted so each 128-node tile has near-uniform degree
    order = np.argsort(-deg, kind="stable")
    perm = np.empty(N_NODES, dtype=np.int64)
    for c in range(NCORES):
        perm[c * NODES_PER_CORE:(c + 1) * NODES_PER_CORE] = order[c::NCORES]
    inv_perm = np.empty(N_NODES, dtype=np.int64)
    inv_perm[perm] = np.arange(N_NODES)

    nsrc = inv_perm[src]
    ndst = inv_perm[dst]

    gp = (nsrc // NODES_PER_CORE) * SLAB + (nsrc % NODES_PER_CORE)
    bank = gp // BANKW
    r16 = (gp - bank * BANKW).astype(np.int64)

    core = ndst // NODES_PER_CORE
    loc = ndst % NODES_PER_CORE
    tile = loc // P
    part = loc % P

    # sort edges by (core, tile, bank, part)
    order_e = np.lexsort((gp, part, bank, tile, core))
    core_s, tile_s, bank_s, part_s = (core[order_e], tile[order_e],
                                      bank[order_e], part[order_e])
    gp_s, r16_s = gp[order_e], r16[order_e]
    g = (((core_s * NTILES + tile_s) * NB + bank_s) * P + part_s)
    ngroups = NCORES * NTILES * NB * P
    cnt = np.bincount(g, minlength=ngroups).reshape(NCORES, NTILES, NB, P)
    rank = _rank_within_groups(g, ngroups)

    # choose K per (tile, bank): tokens gathered via ant path per slot
    Ks = np.arange(0, 28, 4)
    # cost[t,b,K] = ant tokens (always K per slot) + weighted overflow
    c_flat = cnt.transpose(1, 2, 0, 3).reshape(NTILES, NB, NCORES * P)
    costs = np.stack([
        w_ant * Kc * (NCORES * P)
        + w_ind * np.maximum(c_flat - Kc, 0).sum(axis=2)
        for Kc in Ks], axis=2)  # [NT, NB, nK]
    K = Ks[costs.argmin(axis=2)]  # [NT, NB]

    ni_tb = K // 4                           # instrs per (tile, bank)
    n_instr = int(ni_tb.sum())
    instr_base = np.zeros((NTILES, NB), dtype=np.int64)
    run = 0
    bank_of_instr = np.empty(n_instr, dtype=np.int64)
    for t in range(NTILES):
        for b in range(NB):
            instr_base[t, b] = run
            bank_of_instr[run:run + ni_tb[t, b]] = b
            run += ni_tb[t, b]
    cols_total = n_instr * 32

    # ant idx array [NCORES, 16, cols_total] int16, then replicate to 128
    zr = np.asarray(_ZROWS, dtype=np.int16)
    arr = np.empty((NCORES, 16, cols_total), dtype=np.int16)
    arr[:, :, :] = np.repeat(zr[bank_of_instr], 32)[None, None, :]

    Ke = K[tile_s, bank_s]
    sel = rank < Ke
    I = instr_base[tile_s[sel], bank_s[sel]] + rank[sel] // 4
    k = (rank[sel] % 4) * P + part_s[sel]
    c16 = I * 32 + k // 16
    r_in = k % 16
    arr[core_s[sel], r_in, c16] = r16_s[sel].astype(np.int16)
    idx_arr = np.tile(arr, (1, 8, 1))  # [NCORES, 128, cols_total]

    # overflow edges -> per (core, tile, part) column lists
    ov = ~sel
    core_o, tile_o, part_o, gp_o = (core_s[ov], tile_s[ov], part_s[ov],
                                    gp_s[ov])
    g2 = (core_o * NTILES + tile_o) * P + part_o
    # already sorted by (core, tile, bank, part) -> re-sort by (core,tile,part)
    o2 = np.lexsort((g2,))
    g2 = g2[o2]
    gp_o = gp_o[o2]
    part_o2 = part_o[o2]
    tile_o2 = tile_o[o2]
    core_o2 = core_o[o2]
    n2 = NCORES * NTILES * P
    ocnt = np.bincount(g2, minlength=n2).reshape(NCORES, NTILES, P)
    orank = _rank_within_groups(g2, n2)
    D_ov = ocnt.max(axis=(0, 2)).astype(np.int64)  # [NT]

    ov_base = np.concatenate([[0], np.cumsum(P * D_ov)])
    total_ov = int(ov_base[-1])
    offs = np.full((NCORES, max(total_ov, P)), PAD_ROW, dtype=np.int32)
    slot = ov_base[tile_o2] + orank * P + part_o2
    offs[core_o2, slot] = gp_o.astype(np.int32)

    return {
        "perm": perm, "dinv": dinv,
        "K": K, "instr_base": instr_base, "n_instr": n_instr,
        "cols_total": cols_total, "idx_arr": idx_arr,
        "D_ov": D_ov, "ov_base": ov_base, "total_ov": max(total_ov, P),
        "offs": offs,
    }


def _build_program(K, instr_base, n_instr, cols_total, D_ov, ov_base,
                   total_ov):
    import concourse.bass as bass
    import concourse.bacc as bacc
    import concourse.mybir as mybir
    from concourse.tile import TileContext
    from concourse.masks import make_identity
    from concourse import library_config

    fp32 = mybir.dt.float32
    i16 = mybir.dt.int16
    nc = bacc.Bacc("TRN2", target_bir_lowering=False, debug=False,
                   num_devices=NCORES, num_swdge_queues=4)

    g1own = nc.dram_tensor("g1own", [SLAB, FEAT], fp32, kind="ExternalInput").ap()
    idxs_in = nc.dram_tensor("idxs", [P, max(cols_total, 32)], i16,
                             kind="ExternalInput").ap()
    offsets = nc.dram_tensor("offsets", [total_ov], mybir.dt.int32,
                             kind="ExternalInput").ap()
    dinv_in = nc.dram_tensor("dinv", [SLAB, 1], fp32, kind="ExternalInput").ap()
    w2 = nc.dram_tensor("w2", [HID, HID], fp32, kind="ExternalInput").ap()
    w3 = nc.dram_tensor("w3", [HID, HID], fp32, kind="ExternalInput").ap()
    bb = nc.dram_tensor("bb", [P, 3, HID], fp32, kind="ExternalInput").ap()
    h3_out = nc.dram_tensor("h3", [SLAB, HID], fp32, kind="ExternalOutput").ap()

    gown = nc.dram_tensor("gown", [SLAB, ROWW], fp32)
    gfull = nc.dram_tensor("gfull", [GROWS, ROWW], fp32, addr_space="Shared")

    nfull = NODES_PER_CORE // P          # 97 full tiles
    nrem = NODES_PER_CORE - nfull * P    # 84

    # static per-tile gather plans
    TOT = [int(K[t].sum() + D_ov[t]) for t in range(NTILES)]

    # greedy queue assignment: indirect is pinned to q0; ants balance the rest
    ant_cost = 1.0 + 0.0082 * CHUNK
    ind_cost = 1.05
    qload = [0.0, 0.0, 0.0, 0.0]
    qload[0] += ind_cost * float(sum(int(D_ov[t]) for t in range(NTILES)))
    ant_q = []
    for t in range(NTILES):
        for b in range(NB):
            for i in range(int(K[t, b]) // 4):
                qi = min(range(1, 4), key=lambda q: qload[q])
                qload[qi] += ant_cost
                ant_q.append(qi)

    with TileContext(nc) as tc:
        with (
            tc.tile_pool(name="const", bufs=1) as cp,
            tc.tile_pool(name="stageA", bufs=1) as stA,
            tc.tile_pool(name="stageB", bufs=1) as stB,
            tc.tile_pool(name="work", bufs=3) as wp,
            tc.tile_pool(name="small", bufs=6) as sp,
            tc.tile_pool(name="psumT", bufs=2, space="PSUM") as ppT,
            tc.tile_pool(name="psumG", bufs=2, space="PSUM") as ppG,
        ):
            ident = cp.tile([P, P], fp32)
            make_identity(nc, ident[:])
            nc.gpsimd.load_library(library_config.mlp)
            w2t = cp.tile([HID, HID], fp32)
            nc.sync.dma_start(out=w2t[:], in_=w2[:, :])
            w3t = cp.tile([HID, HID], fp32)
            nc.sync.dma_start(out=w3t[:], in_=w3[:, :])
            bbt = cp.tile([P, 3, HID], fp32)
            nc.sync.dma_start(out=bbt[:], in_=bb[:, :, :])
            dinv_t = cp.tile([P, NTILES], fp32)
            nc.sync.dma_start(
                out=dinv_t[:],
                in_=dinv_in[:, 0].rearrange("(t p) -> p t", p=P),
            )
            ncols_ov = total_ov // P
            offs_all = cp.tile([P, ncols_ov], mybir.dt.int32)
            nc.sync.dma_start(
                out=offs_all[:],
                in_=offsets[:].rearrange("(d p) -> p d", p=P),
            )
            idxs_all = cp.tile([P, max(cols_total, 32)], i16)
            nc.sync.dma_start(out=idxs_all[:], in_=idxs_in[:, :])

            # zero gown once (rows >= 12500 and cols >= 30 stay zero forever)
            zt = cp.tile([P, 16, ROWW], fp32)
            nc.vector.memset(zt[:], 0.0)
            for k0 in range(0, NTILES, 16):
                k1 = min(k0 + 16, NTILES)
                nc.sync.dma_start(
                    out=gown[k0 * P:k1 * P, :].rearrange(
                        "(t p) f -> p t f", p=P),
                    in_=zt[:, :k1 - k0, :],
                )

            stage = stA.tile([P, NTILES, HID], fp32)
            stage2 = stB.tile([P, NTILES, HID], fp32)
            nc.sync.dma_start(
                out=stage[:],
                in_=g1own[:, :].rearrange("(t p) f -> p t f", p=P),
            )

            tc.strict_bb_all_engine_barrier()
            r512 = nc.gpsimd.to_reg(CHUNK)

            def publish(st):
                nc.sync.dma_start(
                    out=gown[:nfull * P, :FEAT].rearrange(
                        "(t p) f -> p t f", p=P),
                    in_=st[:, :nfull, :],
                )
                if nrem:
                    nc.sync.dma_start(
                        out=gown[nfull * P:NODES_PER_CORE, :FEAT],
                        in_=st[:nrem, nfull, :],
                    )
                tc.strict_bb_all_engine_barrier()
                nc.gpsimd.collective_compute(
                    "AllGather", mybir.AluOpType.bypass,
                    replica_groups=[list(range(NCORES))],
                    ins=[gown[:, :]], outs=[gfull[:, :]],
                )
                tc.strict_bb_all_engine_barrier()

            publish(stage)

            def compute_tile(t, layer, msg, cur_stage, nxt_stage):
                s0 = sp.tile([P, HID], fp32, tag="s0")
                nc.vector.tensor_reduce(
                    out=s0[:],
                    in_=msg[:, :, :FEAT].rearrange("p d f -> p f d"),
                    axis=mybir.AxisListType.X, op=mybir.AluOpType.add,
                )
                s1 = sp.tile([P, HID], fp32, tag="s1")
                nc.vector.tensor_add(
                    out=s1[:], in0=cur_stage[:, t, :], in1=s0[:])
                s2 = sp.tile([P, HID], fp32, tag="s2")
                nc.vector.scalar_tensor_tensor(
                    out=s2[:], in0=s1[:], scalar=dinv_t[:, t:t + 1],
                    in1=bbt[:, layer, :],
                    op0=mybir.AluOpType.mult, op1=mybir.AluOpType.add,
                )
                h = sp.tile([P, HID], fp32, tag="h")
                nc.scalar.activation(
                    h[:], s2[:], mybir.ActivationFunctionType.Relu)
                if layer < 2:
                    ht_ps = ppT.tile([HID, P], fp32, tag="tps")
                    nc.tensor.transpose(out=ht_ps[:], in_=h[:],
                                        identity=ident[:])
                    ht = sp.tile([HID, P], fp32, tag="ht")
                    nc.vector.tensor_copy(out=ht[:], in_=ht_ps[:])
                    g_ps = ppG.tile([P, HID], fp32, tag="gps")
                    wmat = w2t if layer == 0 else w3t
                    nc.tensor.matmul(out=g_ps[:], lhsT=ht[:], rhs=wmat[:],
                                     start=True, stop=True)
                    nc.vector.tensor_scalar_mul(
                        out=nxt_stage[:, t, :], in0=g_ps[:],
                        scalar1=dinv_t[:, t:t + 1])
                else:
                    nc.vector.tensor_copy(out=nxt_stage[:, t, :], in_=h[:])

            cur_stage, nxt_stage = stage, stage2
            for layer in range(3):
                ant_i = 0
                for t in range(NTILES):
                    msg = wp.tile([P, max(TOT[t], 1), ROWW], fp32,
                                  tag="msg", name="msg")
                    col = 0
                    for b in range(NB):
                        lo = b * BANKW
                        hi = min(lo + BANKW, GROWS)
                        for i in range(int(K[t, b]) // 4):
                            ioff = int(instr_base[t, b] + i) * 32
                            nc.gpsimd.dma_gather(
                                msg[:, col:col + 4, :],
                                gfull[lo:hi, :],
                                idxs_all[:, ioff:ioff + 32],
                                CHUNK, r512, ROWW,
                                single_packet=False,
                                queue_num=ant_q[ant_i],
                            )
                            ant_i += 1
                            col += 4
                    cbase = int(ov_base[t]) // P
                    for j in range(int(D_ov[t])):
                        nc.gpsimd.indirect_dma_start(
                            out=msg[:, col + j, :FEAT],
                            out_offset=None,
                            in_=gfull[:, :],
                            in_offset=bass.IndirectOffsetOnAxis(
                                ap=offs_all[:, cbase + j:cbase + j + 1],
                                axis=0),
                        )
                    compute_tile(t, layer, msg, cur_stage, nxt_stage)
                if layer < 2:
                    publish(nxt_stage)
                cur_stage, nxt_stage = nxt_stage, cur_stage

            nc.sync.dma_start(
                out=h3_out[:, :].rearrange("(t p) f -> p t f", p=P),
                in_=cur_stage[:],
            )

    nc.compile()
    return nc


def kernel(x, edge_index, batch_ids, W1, b1, W2, b2, W3, b3, lin_W, lin_b):
    global _COMPILED, _COMPILED_KEY
    x = np.asarray(x, dtype=np.float32)
    edge_index = np.asarray(edge_index)
    batch_ids = np.asarray(batch_ids)
    W1 = np.asarray(W1, np.float32); b1 = np.asarray(b1, np.float32)
    W2 = np.asarray(W2, np.float32); b2 = np.asarray(b2, np.float32)
    W3 = np.asarray(W3, np.float32); b3 = np.asarray(b3, np.float32)
    lin_W = np.asarray(lin_W, np.float32); lin_b = np.asarray(lin_b, np.float32)

    sched = _build_schedule(edge_index)
    perm, dinv = sched["perm"], sched["dinv"]

    key = (sched["K"].tobytes() + sched["D_ov"].tobytes()
           + np.int64(sched["cols_total"]).tobytes())
    if _COMPILED is None or _COMPILED_KEY != key:
        nc = _build_program(sched["K"], sched["instr_base"],
                            sched["n_instr"], sched["cols_total"],
                            sched["D_ov"], sched["ov_base"],
                            sched["total_ov"])
        _COMPILED = _Runner(nc, NCORES)
        _COMPILED_KEY = key
    r = _COMPILED

    g1 = (x @ W1) * dinv[:, None]
    g1p = g1[perm]
    dinvp = dinv[perm]

    bbc = np.stack([
        np.broadcast_to(b1, (P, HID)),
        np.broadcast_to(b2, (P, HID)),
        np.broadcast_to(b3, (P, HID)),
    ], axis=1).astype(np.float32)  # [P, 3, HID]
    cols_total = sched["cols_total"]
    in_maps = []
    for c in range(NCORES):
        lo, hi = c * NODES_PER_CORE, (c + 1) * NODES_PER_CORE
        g1own = np.zeros((SLAB, FEAT), np.float32)
        g1own[:NODES_PER_CORE] = g1p[lo:hi]
        dv = np.zeros((SLAB, 1), np.float32)
        dv[:NODES_PER_CORE, 0] = dinvp[lo:hi]
        idxs = sched["idx_arr"][c]
        if cols_total < 32:
            pad = np.zeros((P, 32), np.int16)
            pad[:, :cols_total] = idxs
            idxs = pad
        in_maps.append({
            "g1own": g1own,
            "idxs": idxs,
            "offsets": sched["offs"][c],
            "dinv": dv,
            "w2": W2, "w3": W3, "bb": bbc,
        })

    r.put_inputs(in_maps)
    res = r.call()
    results = r.results(res)

    h3p = np.concatenate(
        [results[c]["h3"][:NODES_PER_CORE] for c in range(NCORES)], axis=0)
    h3 = np.empty_like(h3p)
    h3[perm] = h3p
    pooled = np.zeros((N_GRAPHS, HID), np.float32)
    np.add.at(pooled, batch_ids.astype(np.int64), h3)
    return pooled @ lin_W + lin_b
